# revision 2
# baseline (speedup 1.0000x reference)
"""DFine MultiScale Deformable Attention — Trainium2 Bass kernel, v2.

Key change vs v1: the GPSIMD ap_gather was 83% of runtime (~25ns per index,
address-issue bound).  v2 cuts index count 4x:
  - value is packed on the host to bf16 channel-pairs (int32 lanes), so ALL
    256 channels live in one 128-partition table (core j = head j; no
    duplicated head lists across core pairs).
  - the table holds pixel PAIRS (i, i+1) per column (d=2 int32), so one
    index fetches both x-corners (x0, x0+1) of a bilinear patch for 2
    channels.
Per (q, h, p): 2 indices (y0-row pair, y1-row pair) instead of 8
(4 corners x 2 half-tables).  Per batch: 3 gathers x 2688 idx = 8064
index-ops vs 32256.

Pipeline per batch b:
  1. DMA v2p[b] [128, 8448, 2] i32 (host-packed) -> SBUF table T2p.
  2. Frontend in [96=(h,p), 336=q] layout: offsets/attn matmuls, softmax,
     bilinear coords/weights (as v1), w4 [96, 4(t), 336].
  3. Two lin index tiles (y0/y1 rows) -> PE transpose -> lin16q
     [112, sl, h, p, yc] i16 -> DRAM roundtrip -> wrapped idxt [128, 504].
  4. Per slot: ap_gather G [128, 2688, 2] i32 <- T2p.
  5. Weight broadcast via PE selector matmuls (per p), DVE multiply in
     bf16-view of G (weight shared by the channel pair), accumulate over p,
     fold (yc, xc) -> oh [128, slot, sub, 112].
  6. PE-transpose oh back to q-partitions, strided-assemble osb, DMA out.
"""

import os

import numpy as np
import ml_dtypes

import concourse.bass as bass
import concourse.tile as tile
from concourse import bacc, mybir, library_config
from concourse.bass_utils import run_bass_kernel_spmd

F32 = mybir.dt.float32
I16 = mybir.dt.int16
I32 = mybir.dt.int32
BF16 = mybir.dt.bfloat16

# Problem constants
B, LQ, DM, NH, HD = 32, 300, 256, 8, 32
NP_TOT = 12
LVL_W = [80, 40, 20]
LVL_BASE = [0, 6400, 8000]
S = 8400
S_PAD = 8448
N_CORES = 8
BPC = B // N_CORES
SHIFT = 64.0
CAST_BIAS = SHIFT - 1.0
REPEAT = int(os.environ.get("BASS_REPEAT", "1"))

QP, NSLOT, Q336 = 112, 3, 336
KSLOT = [7, 7, 5]                   # k-blocks per slot (slot2 tail is pad)
# idx cols per slot: c = k*24 + p*2 + yc  (168 cols); num_idxs = 16*168
IDXC_SLOT = 168
NIDX_SLOT = 16 * IDXC_SLOT          # 2688


def _wl(p):
    return float(LVL_W[p // 4])


def _base(p):
    return float(LVL_BASE[p // 4])


def make_consts():
    c = {}
    c["ident"] = np.eye(128, dtype=np.float32)
    # selp2: 12 matrices [96, 128]; selp2[h*12+p, p*128 + (h*16 + d)] = 1
    selp2 = np.zeros((96, 12 * 128), np.float32)
    for h in range(NH):
        for p in range(12):
            for d in range(16):
                selp2[h * 12 + p, p * 128 + h * 16 + d] = 1.0
    c["selp2"] = selp2
    sel8 = np.zeros((96, 8), np.float32)
    rep8 = np.zeros((8, 96), np.float32)
    for h in range(8):
        for p in range(12):
            sel8[h * 12 + p, h] = 1.0
            rep8[h, h * 12 + p] = 1.0
    c["sel8"] = sel8
    c["rep8"] = rep8
    selr = np.zeros((4, 4 * 96), np.float32)
    for hp in range(96):
        p = hp % 12
        w = _wl(p)
        selr[0, 0 * 96 + hp] = w
        selr[1, 1 * 96 + hp] = w
        selr[2, 2 * 96 + hp] = 0.125 * w
        selr[3, 3 * 96 + hp] = 0.125 * w
    c["selr"] = selr
    c["ones1"] = np.ones((1, Q336), np.float32)
    cv = np.zeros((96, 8), np.float32)
    for hp in range(96):
        p = hp % 12
        w, base = _wl(p), _base(p)
        cv[hp, 0] = w - 1.0 + SHIFT                   # XMAX
        cv[hp, 1] = w - 2.0 + SHIFT                   # XM63
        cv[hp, 2] = w                                 # Wv
        cv[hp, 3] = base - SHIFT * w - SHIFT + 1.0          # ClA (y0 row)
        cv[hp, 5] = base - (SHIFT - 1.0) * w - SHIFT + 1.0  # ClB (y1 row)
    c["cv"] = cv
    return c


def pack_value(value):
    """[B, 8400, 256] f32 -> [B, 128, 8448, 2] int32 (bf16 ch-pair, pixel
    pair (i, i+1)); partition cp = h*16 + d covers channels (2cp, 2cp+1)."""
    Bn = value.shape[0]
    v16 = np.asarray(value, dtype=ml_dtypes.bfloat16).view(np.uint16)
    vp = np.zeros((Bn, S_PAD + 8, DM), np.uint16)
    vp[:, 1:S + 1] = v16
    pk = vp[:, :, 0::2].astype(np.uint32) | (vp[:, :, 1::2].astype(np.uint32) << 16)
    v2 = np.stack([pk[:, 0:S_PAD, :], pk[:, 1:S_PAD + 1, :]], axis=-1)
    return np.ascontiguousarray(v2.transpose(0, 2, 1, 3)).view(np.int32)


def emit(nc):
    v2p = nc.dram_tensor("v2p", [BPC, 128, S_PAD * 2], I32, kind="ExternalInput").ap()
    query = nc.dram_tensor("query", [BPC, LQ, DM], F32, kind="ExternalInput").ap()
    refp = nc.dram_tensor("refp", [BPC, LQ, 4], F32, kind="ExternalInput").ap()
    woff = nc.dram_tensor("woff", [DM, 192], F32, kind="ExternalInput").ap()
    wattn = nc.dram_tensor("wattn", [DM, 96], F32, kind="ExternalInput").ap()
    boff = nc.dram_tensor("boff", [1, 192], F32, kind="ExternalInput").ap()
    battn = nc.dram_tensor("battn", [1, 96], F32, kind="ExternalInput").ap()
    ident_d = nc.dram_tensor("ident", [128, 128], F32, kind="ExternalInput").ap()
    selp2_d = nc.dram_tensor("selp2", [96, 12 * 128], F32, kind="ExternalInput").ap()
    sel8_d = nc.dram_tensor("sel8", [96, 8], F32, kind="ExternalInput").ap()
    rep8_d = nc.dram_tensor("rep8", [8, 96], F32, kind="ExternalInput").ap()
    selr_d = nc.dram_tensor("selr", [4, 4 * 96], F32, kind="ExternalInput").ap()
    ones1_d = nc.dram_tensor("ones1", [1, Q336], F32, kind="ExternalInput").ap()
    cv_d = nc.dram_tensor("cv", [96, 8], F32, kind="ExternalInput").ap()
    out_d = nc.dram_tensor("out", [BPC, LQ, DM], F32, kind="ExternalOutput").ap()
    linq_d = nc.dram_tensor("linq", [BPC, QP, NSLOT * 8 * 12 * 2], I16,
                            kind="Internal").ap()

    MUL, ADD, SUB, MAX, MIN, EQ = (
        mybir.AluOpType.mult, mybir.AluOpType.add, mybir.AluOpType.subtract,
        mybir.AluOpType.max, mybir.AluOpType.min, mybir.AluOpType.is_equal)
    EXP = mybir.ActivationFunctionType.Exp

    with tile.TileContext(nc) as tc:
        import contextlib
        ctx = contextlib.ExitStack()
        with ctx:
            cpool = ctx.enter_context(tc.tile_pool(name="consts", bufs=1))
            tpool = ctx.enter_context(tc.tile_pool(name="tables", bufs=1))
            gpool = ctx.enter_context(tc.tile_pool(name="gath", bufs=2))
            fpool = ctx.enter_context(tc.tile_pool(name="front", bufs=16))
            wpool = ctx.enter_context(tc.tile_pool(name="w4", bufs=1))
            ppool = ctx.enter_context(tc.tile_pool(name="ptree", bufs=4))
            opool = ctx.enter_context(tc.tile_pool(name="outsb", bufs=1))
            ipool = ctx.enter_context(tc.tile_pool(name="idx", bufs=2))
            qpool = ctx.enter_context(tc.tile_pool(name="qt", bufs=1))
            psS = ctx.enter_context(tc.tile_pool(name="psS", bufs=1, space="PSUM"))
            psA = ctx.enter_context(tc.tile_pool(name="psA", bufs=1, space="PSUM"))
            psW = ctx.enter_context(tc.tile_pool(name="psW", bufs=2, space="PSUM"))
            psF = ctx.enter_context(tc.tile_pool(name="psF", bufs=1, space="PSUM"))

            nc.gpsimd.load_library(library_config.ap_gather)

            def ld(dst, src):
                nc.sync.dma_start(dst, src)

            ident = cpool.tile([128, 128], F32, name="ident")
            ld(ident[:], ident_d)
            selp2 = cpool.tile([96, 12 * 128], F32, name="selp2")
            ld(selp2[:], selp2_d)
            sel8 = cpool.tile([96, 8], F32, name="sel8")
            ld(sel8[:], sel8_d)
            rep8 = cpool.tile([8, 96], F32, name="rep8")
            ld(rep8[:], rep8_d)
            selr = cpool.tile([4, 4 * 96], F32, name="selr")
            ld(selr[:], selr_d)
            ones1 = cpool.tile([1, Q336], F32, name="ones1")
            ld(ones1[:], ones1_d)
            cv = cpool.tile([96, 8], F32, name="cv")
            ld(cv[:], cv_d)
            woff_sb = cpool.tile([128, 2, 192], F32, name="woff_sb")
            ld(woff_sb[:], woff.rearrange("(kt p) m -> p kt m", p=128))
            wattn_sb = cpool.tile([128, 2, 96], F32, name="wattn_sb")
            ld(wattn_sb[:], wattn.rearrange("(kt p) m -> p kt m", p=128))
            boff_sb = cpool.tile([1, 192], F32, name="boff_sb")
            ld(boff_sb[:], boff)
            battn_sb = cpool.tile([1, 96], F32, name="battn_sb")
            ld(battn_sb[:], battn)

            def cvs(k):
                return cv[:, k:k + 1]

            def fs(nm):
                return fpool.tile([96, Q336], F32, name=nm, tag="fs")

            for b4 in range(BPC * REPEAT):
                b = b4 % BPC
                # ==== 1. value table ====================================
                T2p = tpool.tile([128, S_PAD, 2], I32, name="T2p", tag="T2p")
                nc.sync.dma_start(
                    T2p[:].rearrange("c i d -> c (i d)"), v2p[b])

                # ==== 2. frontend =======================================
                qsb = qpool.tile([QP, NSLOT, DM], F32, name="qsb")
                nc.vector.memset(qsb[64:112, 2, :], 0.0)
                nc.sync.dma_start(
                    qsb[:, 0:2, :],
                    query[b][0:224].rearrange("(s r) c -> r s c", r=QP))
                nc.sync.dma_start(qsb[0:76, 2, :], query[b][224:300, :])
                refsb = qpool.tile([QP, NSLOT, 4], F32, name="refsb")
                nc.vector.memset(refsb[64:112, 2, :], 0.0)
                nc.sync.dma_start(
                    refsb[:, 0:2, :],
                    refp[b][0:224].rearrange("(s r) c -> r s c", r=QP))
                nc.sync.dma_start(refsb[0:76, 2, :], refp[b][224:300, :])

                qT = [qpool.tile([128, Q336], F32, name=f"qT{kt}") for kt in range(2)]
                for slot in range(NSLOT):
                    for kt in range(2):
                        pt = psS.tile([128, 128], F32, name="pss", tag="pss")
                        nc.tensor.transpose(
                            pt[:, 0:QP], qsb[:, slot, kt * 128:(kt + 1) * 128],
                            ident[0:QP, 0:QP])
                        nc.scalar.copy(qT[kt][:, slot * QP:(slot + 1) * QP],
                                       pt[:, 0:QP])
                refT = fs("refT")
                for slot in range(NSLOT):
                    pt = psS.tile([128, 128], F32, name="pss", tag="pss")
                    nc.tensor.transpose(pt[0:4, 0:QP], refsb[:, slot, :],
                                        ident[0:QP, 0:QP])
                    nc.scalar.copy(refT[0:4, slot * QP:(slot + 1) * QP],
                                   pt[0:4, 0:QP])

                refb = []
                for m in range(4):
                    ps = psF.tile([96, Q336], F32, name="psf", tag="psf")
                    nc.tensor.matmul(ps[:], selr[:, m * 96:(m + 1) * 96], refT[0:4, :])
                    sb = fs(f"refb{m}")
                    nc.scalar.copy(sb[:], ps[:])
                    refb.append(sb)
                rxw, ryw, rwc, rhc = refb

                def head_mm(w_sb, bias_sb, cols, nm):
                    ps = psF.tile([96, Q336], F32, name="psf", tag="psf")
                    nc.tensor.matmul(ps[:], w_sb[:, 0, cols], qT[0][:],
                                     start=True, stop=False)
                    nc.tensor.matmul(ps[:], w_sb[:, 1, cols], qT[1][:],
                                     start=False, stop=False)
                    nc.tensor.matmul(ps[:], bias_sb[:, cols], ones1[:],
                                     start=False, stop=True)
                    return ps

                logit_ps = head_mm(wattn_sb, battn_sb, slice(0, 96), "logit")
                expT = fs("expT")
                nc.scalar.activation(expT[:], logit_ps[:], EXP)
                s_ps = psF.tile([96, Q336], F32, name="psf", tag="psf")
                nc.tensor.matmul(s_ps[0:8, :], sel8[:], expT[:])
                rsum = fs("rsum")
                nc.vector.reciprocal(rsum[0:8, :], s_ps[0:8, :])
                rb_ps = psF.tile([96, Q336], F32, name="psf", tag="psf")
                nc.tensor.matmul(rb_ps[:], rep8[:], rsum[0:8, :])
                attnT = fs("attnT")
                nc.vector.tensor_tensor(attnT[:], expT[:], rb_ps[:], MUL)

                offx_ps = head_mm(woff_sb, boff_sb, slice(0, 192, 2), "offx")
                offx = fs("offx")
                nc.scalar.copy(offx[:], offx_ps[:])
                offy_ps = head_mm(woff_sb, boff_sb, slice(1, 192, 2), "offy")
                offy = fs("offy")
                nc.scalar.copy(offy[:], offy_ps[:])

                def coord(off_sb, rXw, rWc, sfx):
                    t1 = fs("t1" + sfx)
                    nc.vector.tensor_tensor(t1[:], off_sb[:], rWc[:], MUL)
                    ixp = fs("ixp" + sfx)
                    nc.vector.scalar_tensor_tensor(
                        ixp[:], t1[:], CAST_BIAS, rXw[:], ADD, ADD)
                    fi = fpool.tile([96, Q336], I16, name="fi" + sfx, tag="fi")
                    nc.vector.tensor_copy(fi[:], ixp[:])
                    fxp = fs("fxp" + sfx)
                    nc.vector.tensor_copy(fxp[:], fi[:])
                    a0 = fs("a0" + sfx)
                    nc.vector.tensor_scalar(a0[:], fxp[:], SHIFT, cvs(0), MAX, MIN)
                    a1 = fs("a1" + sfx)
                    nc.vector.tensor_scalar(a1[:], fxp[:], SHIFT - 1.0, cvs(1), MAX, MIN)
                    v0 = fs("v0" + sfx)
                    nc.vector.tensor_tensor(v0[:], a0[:], fxp[:], EQ)
                    v1 = fs("v1" + sfx)
                    nc.vector.tensor_tensor(v1[:], a1[:], fxp[:], EQ)
                    fx = fs("fx" + sfx)
                    nc.vector.scalar_tensor_tensor(
                        fx[:], ixp[:], SHIFT - CAST_BIAS - 0.5, fxp[:], ADD, SUB)
                    omf = fs("omf" + sfx)
                    nc.vector.tensor_scalar(omf[:], fx[:], -1.0, 1.0, MUL, ADD)
                    w0 = fs("w0" + sfx)
                    nc.vector.tensor_tensor(w0[:], omf[:], v0[:], MUL)
                    w1 = fs("w1" + sfx)
                    nc.vector.tensor_tensor(w1[:], fx[:], v1[:], MUL)
                    bp = fs("bp" + sfx)
                    nc.vector.tensor_scalar(bp[:], fxp[:], SHIFT - 1.0, cvs(0),
                                            MAX, MIN)
                    return a0, a1, w0, w1, bp

                a0x, a1x, wx0, wx1, bpx = coord(offx, rxw, rwc, "x")
                a0y, a1y, wy0, wy1, _bpy = coord(offy, ryw, rhc, "y")

                wy0a = fs("wy0a")
                nc.vector.tensor_tensor(wy0a[:], wy0[:], attnT[:], MUL)
                wy1a = fs("wy1a")
                nc.vector.tensor_tensor(wy1a[:], wy1[:], attnT[:], MUL)

                # w4 [96, 4, 336]: t = 2*yc + xc
                w4 = wpool.tile([96, 4, Q336], F32, name="w4")
                nc.vector.tensor_tensor(w4[:, 0, :], wy0a[:], wx0[:], MUL)
                nc.vector.tensor_tensor(w4[:, 1, :], wy0a[:], wx1[:], MUL)
                nc.vector.tensor_tensor(w4[:, 2, :], wy1a[:], wx0[:], MUL)
                nc.vector.tensor_tensor(w4[:, 3, :], wy1a[:], wx1[:], MUL)

                # ==== 3. lin indices (2 per (q,h,p): y0/y1 row pairs) ====
                yw0 = fs("yw0")
                nc.vector.tensor_scalar(yw0[:], a0y[:], cvs(2), None, MUL)
                yw1 = fs("yw1")
                nc.vector.tensor_scalar(yw1[:], a1y[:], cvs(2), None, MUL)
                lin16q = ipool.tile([QP, NSLOT, 8, 12, 2], I16, name="lin16q",
                                    bufs=1)
                for yc, (cl, yw) in enumerate(((3, yw0), (5, yw1))):
                    lf = fs("linf")
                    nc.vector.scalar_tensor_tensor(lf[:], bpx[:], cvs(cl), yw[:],
                                                   ADD, ADD)
                    for slot in range(NSLOT):
                        pt = psS.tile([128, 128], F32, name="pss", tag="pss")
                        nc.tensor.transpose(
                            pt[0:QP, 0:96], lf[:, slot * QP:(slot + 1) * QP],
                            ident[0:96, 0:96])
                        dst = lin16q[:, slot, :, :, yc].rearrange(
                            "r h p -> r (h p)")
                        nc.scalar.copy(dst, pt[0:QP, 0:96])

                # wrap via DRAM roundtrip -> idxt [128=(16h+a), 504]
                nc.sync.dma_start(
                    linq_d[b], lin16q[:].rearrange("r sl h p yc -> r (sl h p yc)"))
                idxt = ipool.tile([128, NSLOT * IDXC_SLOT], I16, name="idxt",
                                  tag="idxt")
                lsrc = linq_d[b].rearrange(
                    "(k a) (sl h c) -> h a k sl c", a=16, sl=NSLOT, h=8)
                for sl in range(NSLOT):
                    for k in range(KSLOT[sl]):
                        c0 = sl * IDXC_SLOT + k * 24
                        nc.sync.dma_start(idxt[:, c0:c0 + 24],
                                          lsrc[:, :, k, sl, :])

                # ==== 4+5. gather + weighted reduce =====================
                oh = opool.tile([128, NSLOT, 2, QP], F32, name="oh")
                nc.vector.memset(oh[:, 2, :, 80:112], 0.0)
                for slot in range(NSLOT):
                    nk = KSLOT[slot]
                    nidx = 16 * nk * 24
                    G = gpool.tile([128, NIDX_SLOT, 2], I32, name="G", tag="G")
                    # sub-gathers of <=1152 idx: ap_gather cost/idx
                    # roughly doubles past ~1400 idx per call
                    ksplit = [(0, 3), (3, 3), (6, 1)] if nk == 7 else \
                             [(0, 3), (3, 2)]
                    for k0, kn in ksplit:
                        nc.gpsimd.ap_gather(
                            G[:, k0 * 384:(k0 + kn) * 384, :], T2p[:],
                            idxt[:, slot * IDXC_SLOT + k0 * 24:
                                 slot * IDXC_SLOT + (k0 + kn) * 24],
                            channels=128, num_elems=S_PAD, d=2,
                            num_idxs=kn * 384)
                    # bf16 view: free = (k, p, yc, rest=(a, xc, sub))
                    Gv = G[:, 0:nidx, :].rearrange("c i d -> c (i d)").bitcast(
                        BF16).rearrange(
                        "c (k p yc r) -> c k p yc r", k=nk, p=12, yc=2)
                    rhs = w4[:, :, slot * QP:slot * QP + nk * 16].rearrange(
                        "c (yc xc) (k a) -> c yc k a xc", yc=2, k=nk)
                    accs = [None, None, None, None]
                    for p in range(12):
                        wp = [psW.tile([128, 7 * 32], F32, name=f"wp{yc}",
                                       tag=f"wp{yc}") for yc in range(2)]
                        for yc in range(2):
                            nc.tensor.matmul(
                                wp[yc][:, 0:nk * 32],
                                selp2[:, p * 128:(p + 1) * 128],
                                rhs[:, yc])
                        for s in range(2):
                            for yc in range(2):
                                gsl = Gv[:, :, p, yc, s::2]
                                wpv = wp[yc][:, 0:nk * 32].rearrange(
                                    "c (k r) -> c k r", k=nk)
                                nc.vector.tensor_tensor(gsl, gsl, wpv, MUL)
                                if p == 0:
                                    accs[s * 2 + yc] = gsl
                                elif p == 1:
                                    # accumulate in PSUM: frees the shared
                                    # SBUF port for GPSIMD gathers
                                    na = psA.tile([128, 2, 7, 32], F32,
                                                  name=f"accY{yc}",
                                                  tag=f"accY{yc}")
                                    nc.vector.tensor_tensor(
                                        na[:, s, 0:nk, :], accs[s * 2 + yc],
                                        gsl, ADD)
                                    accs[s * 2 + yc] = na[:, s, 0:nk, :]
                                else:
                                    nc.vector.tensor_tensor(
                                        accs[s * 2 + yc],
                                        accs[s * 2 + yc], gsl, ADD)
                    for s in range(2):
                        # PSUM->SBUF copies on ACT (DVE cannot read 2 PSUM ins)
                        asb = [ppool.tile([128, 7, 32], F32, name=f"asb{s}{yc}",
                                          tag=f"asb{s}{yc}") for yc in range(2)]
                        for yc in range(2):
                            nc.scalar.copy(asb[yc][:, 0:nk, :],
                                           accs[s * 2 + yc])
                        x2 = ppool.tile([128, 7, 32], F32, name=f"x2{s}",
                                        tag=f"x2{s}")
                        nc.vector.tensor_tensor(
                            x2[:, 0:nk, :], asb[0][:, 0:nk, :],
                            asb[1][:, 0:nk, :], ADD)
                        nc.vector.tensor_tensor(
                            oh[:, slot, s, 0:nk * 16].rearrange(
                                "c (k a) -> c k a", k=nk),
                            x2[:, 0:nk, 0::2], x2[:, 0:nk, 1::2], ADD)

                # ==== 6. output transpose + store =======================
                osb = opool.tile([QP, NSLOT, DM], F32, name="osb")
                for slot in range(NSLOT):
                    for s in range(2):
                        pt = psS.tile([128, 128], F32, name="pss", tag="pss")
                        nc.tensor.transpose(
                            pt[0:QP, :], oh[:, slot, s, :], ident[:])
                        nc.scalar.copy(
                            osb[:, slot, :].rearrange("r (cp s2) -> r s2 cp",
                                                      s2=2)[:, s, :],
                            pt[0:QP, :])

                nc.sync.dma_start(
                    out_d[b][0:224].rearrange("(s r) c -> r s c", r=QP),
                    osb[:, 0:2, :])
                nc.sync.dma_start(out_d[b][224:300, :], osb[0:76, 2, :])
    return nc


_CACHE = {}


def _get_nc():
    key = ("nc", REPEAT)
    if key not in _CACHE:
        nc = bacc.Bacc("TRN2", target_bir_lowering=False, debug=False,
                       enable_asserts=False)
        emit(nc)
        nc.compile()
        _CACHE[key] = nc
    return _CACHE[key]


def _in_maps(query, reference_points, input_flatten, W_off, b_off, W_attn,
             b_attn):
    query = np.asarray(query, np.float32)
    refp = np.asarray(reference_points, np.float32).reshape(B, LQ, 4)
    v2p = pack_value(np.asarray(input_flatten, np.float32))
    v2p = v2p.reshape(B, 128, S_PAD * 2)
    consts = make_consts()
    in_maps = []
    for c in range(N_CORES):
        sl = slice(c * BPC, (c + 1) * BPC)
        in_maps.append({
            "v2p": v2p[sl], "query": query[sl], "refp": refp[sl],
            "woff": np.asarray(W_off, np.float32),
            "wattn": np.asarray(W_attn, np.float32),
            "boff": np.asarray(b_off, np.float32).reshape(1, 192),
            "battn": np.asarray(b_attn, np.float32).reshape(1, 96),
            **{k: v for k, v in consts.items()},
        })
    return in_maps


def kernel(query, reference_points, input_flatten, W_off, b_off, W_attn, b_attn):
    in_maps = _in_maps(query, reference_points, input_flatten, W_off, b_off,
                       W_attn, b_attn)
    res = run_bass_kernel_spmd(_get_nc(), in_maps, core_ids=list(range(N_CORES)))
    out = np.concatenate([r["out"] for r in res.results], axis=0)
    return out.astype(np.float32)


if __name__ == "__main__":
    import reference
    inputs = reference.setup_inputs()
    inputs = {k: np.asarray(v) for k, v in inputs.items()}
    got = kernel(**inputs)
    exp = np.asarray(reference.reference(**inputs))
    err = np.abs(got - exp).max() / np.abs(exp).max()
    print("Relative error:", err)
